# revision 1
# baseline (speedup 1.0000x reference)
"""Trainium2 Bass kernel for LocalSelfAttentionHeadSum.

Reference computation (per sample b of B=32):
  x = x_window[b] reshaped (C=1024, THW=1764); x_item = cols 784:980 (center frame)
  q = Wq @ x_item + bq          (512, 196)
  k = Wk @ x + bk               (512, 1764)
  v = Wv @ x + bv               (512, 1764)
  alpha = softmax(q^T k, axis=-1)
  y = v @ alpha^T               (512, 196)
  out = Wo @ y + bo             (1024, 196)

Sharding: data-parallel over B across 8 cores (4 samples per core).

Kernel structure per core:
  - DMA order chosen so the PE starts after ~4MB lands (Wk hi + first x chunk):
    the first K/V projection (sample 0, chunk 0) is emitted before the batched
    Q projection.
  - Q-projection batched over the 4 samples (moving dim 784 -> full-rate tf32).
  - Per sample, stream key-chunks [512, 512, 370, 370]:
      K-proj (weights stationary) -> scores S = q^T k in (query-part, key-free)
      layout -> exp on ScalarE with fused row-sum (softmax denominator; no max
      subtraction needed: |S| <~ 60 keeps exp inside fp32 range) -> PE-transpose
      exp(S) -> AV matmul accumulated in PSUM across the 14 key subtiles.
      V-proj produces v^T directly (x stationary, Wv^T moving).
  - normalize by 1/Z, PE-transpose y^T -> y, add bv.
  - output projection batched over samples, + bo, DMA out.

Precision: fp32r (tf32) matmuls at full PE rate (1 cyc/row for moving dim
>=256); Wq/Wk applied in two hi/lo tf32 passes (host-split) because score
errors are amplified ~20x by the |q||k|/|S| cancellation + exp.
"""

import os
import numpy as np

import concourse.bass as bass
import concourse.tile as tile
from concourse import bacc, mybir
from concourse.bass_utils import run_bass_kernel_spmd
from concourse.masks import make_identity

F32 = mybir.dt.float32
F32R = mybir.dt.float32r

# Problem shapes (hardcoded per contract)
B, C, T, H, W = 32, 1024, 9, 14, 14
CI = 512
HW = H * W              # 196
THW = T * HW            # 1764
NCORES = 8
BPC = B // NCORES       # 4 samples per core
CT = C // 128           # 8 C-tiles
MI = CI // 128          # 4 Ci-tiles
ITEM0 = (T // 2) * HW   # 784, center-frame column offset
QT = [(0, 128), (128, HW - 128)]          # query partition tiles
CHUNKS = [(0, 512), (512, 512), (1024, 370), (1394, 370)]
SAMP = BPC * HW         # 784 batched (sample, query) columns
NCH = [(0, 512), (512, SAMP - 512)]       # batched free-dim chunks

NPASS_Q = int(os.environ.get("NPASS_Q", "2"))
NPASS_K = int(os.environ.get("NPASS_K", "2"))
NPASS_V = int(os.environ.get("NPASS_V", "1"))
NPASS_O = int(os.environ.get("NPASS_O", "1"))
REPEAT = int(os.environ.get("KREPEAT", "1"))


def _subtiles(n):
    out = []
    o = 0
    while o < n:
        out.append((o, min(128, n - o)))
        o += 128
    return out


def build_kernel():
    nc = bacc.Bacc("TRN2", target_bir_lowering=False, debug=False)

    x_d = nc.dram_tensor("x", [BPC, C, THW], F32, kind="ExternalInput")
    wqt_d = [nc.dram_tensor(f"wqt{i}", [C, CI], F32, kind="ExternalInput")
             for i in range(NPASS_Q)]
    wkt_d = [nc.dram_tensor(f"wkt{i}", [C, CI], F32, kind="ExternalInput")
             for i in range(NPASS_K)]
    wvt_d = [nc.dram_tensor(f"wvt{i}", [C, CI], F32, kind="ExternalInput")
             for i in range(NPASS_V)]
    wot_d = [nc.dram_tensor(f"wot{i}", [CI, C], F32, kind="ExternalInput")
             for i in range(NPASS_O)]
    bq_d = nc.dram_tensor("bq", [MI, 128], F32, kind="ExternalInput")
    bk_d = nc.dram_tensor("bk", [MI, 128], F32, kind="ExternalInput")
    bv_d = nc.dram_tensor("bv", [MI, 128], F32, kind="ExternalInput")
    bo_d = nc.dram_tensor("bo", [CT, 128], F32, kind="ExternalInput")
    out_d = nc.dram_tensor("out", [BPC, C, HW], F32, kind="ExternalOutput")

    with tile.TileContext(nc) as tc:
        with tc.tile_pool(name="const", bufs=1) as const_pool:
            ident = const_pool.tile([128, 128], F32)
            bq_sb = const_pool.tile([128, MI], F32)
            bk_sb = const_pool.tile([128, MI], F32)
            bv_sb = const_pool.tile([128, MI], F32)
            bo_sb = const_pool.tile([128, CT], F32)

            env = dict(
                nc=nc, tc=tc, x_d=x_d, out_d=out_d,
                wqt_d=wqt_d, wkt_d=wkt_d, wvt_d=wvt_d, wot_d=wot_d,
                bq_d=bq_d, bk_d=bk_d, bv_d=bv_d, bo_d=bo_d,
                ident=ident, bq_sb=bq_sb, bk_sb=bk_sb, bv_sb=bv_sb, bo_sb=bo_sb,
            )
            for rep in range(REPEAT):
                _emit_iteration(env, first=(rep == 0))

    nc.compile()
    return nc


def _emit_iteration(env, first):
    nc, tc = env["nc"], env["tc"]
    x_d, out_d = env["x_d"], env["out_d"]
    ident = env["ident"]
    bq_sb, bk_sb, bv_sb, bo_sb = (env["bq_sb"], env["bk_sb"],
                                  env["bv_sb"], env["bo_sb"])

    persist = tc.alloc_tile_pool(name="persist", bufs=1)
    pA = tc.alloc_tile_pool(name="phaseA", bufs=1, side="right")
    xc_pool = tc.alloc_tile_pool(name="xc", bufs=3)
    k_pool = tc.alloc_tile_pool(name="ksb", bufs=2)
    vt_pool = tc.alloc_tile_pool(name="vtsb", bufs=5)
    pacc = tc.alloc_tile_pool(name="pacc", bufs=3, space="PSUM")
    pS_pool = tc.alloc_tile_pool(name="pS", bufs=1, space="PSUM")
    pT_pool = tc.alloc_tile_pool(name="pT", bufs=2, space="PSUM")
    pY_pool = tc.alloc_tile_pool(name="pY", bufs=2, space="PSUM")
    late = {}

    # ---- DMA emission order is the startup-latency lever ----
    wkt = [persist.tile([128, CT, CI], F32R, tag=f"wkt{i}", name=f"wkt{i}")
           for i in range(NPASS_K)]
    wvt = [persist.tile([128, CT, CI], F32R, tag=f"wvt{i}", name=f"wvt{i}")
           for i in range(NPASS_V)]
    wqt = [pA.tile([128, CT, CI], F32R, tag=f"wqt{i}", name=f"wqt{i}")
           for i in range(NPASS_Q)]

    def dma_w(dst, src_d):
        for t in range(dst.shape[1]):
            p = 128
            nc.sync.dma_start(
                dst[:, t, :],
                src_d[t * p:(t + 1) * p, :].rearrange("p e -> p e").bitcast(F32R))

    if first:
        nc.sync.dma_start(bq_sb[:], env["bq_d"][:].rearrange("m p -> p m"))
        nc.sync.dma_start(bk_sb[:], env["bk_d"][:].rearrange("m p -> p m"))
        nc.sync.dma_start(bv_sb[:], env["bv_d"][:].rearrange("m p -> p m"))
        nc.sync.dma_start(bo_sb[:], env["bo_d"][:].rearrange("m p -> p m"))
    dma_w(wkt[0], env["wkt_d"][0])

    def dma_x_chunk(s, ci):
        c0, csz = CHUNKS[ci]
        x_c = xc_pool.tile([128, CT, 512], F32R, tag="xc", name=f"xc{s}_{ci}")
        for t in range(CT):
            nc.sync.dma_start(
                x_c[:, t, :csz],
                x_d[s, t * 128:(t + 1) * 128, c0:c0 + csz].bitcast(F32R))
        return x_c

    x_c00 = dma_x_chunk(0, 0)
    for i in range(1, NPASS_K):
        dma_w(wkt[i], env["wkt_d"][i])
    for i in range(NPASS_V):
        dma_w(wvt[i], env["wvt_d"][i])
    x_c01 = dma_x_chunk(0, 1)
    x_c02 = dma_x_chunk(0, 2)
    if first:
        make_identity(nc, ident[:])

    # ---- per-chunk building blocks ----
    def kv_proj(s, ci, x_c):
        c0, csz = CHUNKS[ci]
        k_sb = k_pool.tile([128, MI, 512], F32R, tag="ksb", name=f"ksb{s}_{ci}")
        for m in range(MI):
            pk = pacc.tile([128, 512], F32, tag="acc", name=f"pk{s}_{ci}_{m}")
            for ip in range(NPASS_K):
                for t in range(CT):
                    nc.tensor.matmul(
                        pk[:, :csz],
                        wkt[ip][:, t, m * 128:(m + 1) * 128],
                        x_c[:, t, :csz],
                        start=(t == 0 and ip == 0),
                        stop=(t == CT - 1 and ip == NPASS_K - 1))
            nc.scalar.activation(
                k_sb[:, m, :csz], pk[:, :csz],
                mybir.ActivationFunctionType.Identity,
                bias=bk_sb[:, m:m + 1])
        vts = []
        for kj, (ko, ksz) in enumerate(_subtiles(csz)):
            pv = pacc.tile([128, CI], F32, tag="acc", name=f"pv{s}_{ci}_{kj}")
            for ip in range(NPASS_V):
                for t in range(CT):
                    nc.tensor.matmul(
                        pv[:ksz, :],
                        x_c[:, t, ko:ko + ksz],
                        wvt[ip][:, t, :],
                        start=(t == 0 and ip == 0),
                        stop=(t == CT - 1 and ip == NPASS_V - 1))
            vt = vt_pool.tile([128, CI], F32R, tag="vt", name=f"vt{s}_{ci}_{kj}")
            nc.vector.tensor_copy(vt[:ksz, :], pv[:ksz, :])
            vts.append(vt)
        return k_sb, vts

    def attn_chunk(s, ci, k_sb, vts, q_all, py, zs, kt_base, n_kt_total):
        c0, csz = CHUNKS[ci]
        subs = _subtiles(csz)
        ets = [late["et"].tile([128, HW], F32R, tag="et", name=f"et{s}_{ci}_{kj}")
               for kj in range(len(subs))]
        for qi, (qo, qsz) in enumerate(QT):
            ps = pS_pool.tile([128, 512], F32, tag="ps", name=f"ps{s}_{ci}_{qi}")
            for m in range(MI):
                nc.tensor.matmul(
                    ps[:qsz, :csz],
                    q_all[:, m, s * HW + qo: s * HW + qo + qsz],
                    k_sb[:, m, :csz],
                    start=(m == 0), stop=(m == MI - 1))
            e_sb = late["e"].tile([128, 512], F32, tag="e", name=f"e{s}_{ci}_{qi}")
            nc.scalar.activation(
                e_sb[:qsz, :csz], ps[:qsz, :csz],
                mybir.ActivationFunctionType.Exp,
                accum_out=zs[qi][:qsz, ci:ci + 1])
            for kj, (ko, ksz) in enumerate(subs):
                pe = pT_pool.tile([128, 128], F32, tag="pt",
                                  name=f"pe{s}_{ci}_{qi}_{kj}")
                nc.tensor.transpose(
                    pe[:ksz, :qsz], e_sb[:qsz, ko:ko + ksz], ident[:qsz, :qsz])
                nc.vector.tensor_copy(ets[kj][:ksz, qo:qo + qsz], pe[:ksz, :qsz])
        for kj, (ko, ksz) in enumerate(subs):
            for qi, (qo, qsz) in enumerate(QT):
                nc.tensor.matmul(
                    py[qi][:qsz, :],
                    ets[kj][:ksz, qo:qo + qsz],
                    vts[kj][:ksz, :],
                    start=(kt_base + kj == 0),
                    stop=(kt_base + kj == n_kt_total - 1))

    # ---- first chunks' projections run during the weight/x_item DMA tail ----
    kv00 = kv_proj(0, 0, x_c00)
    kv01 = kv_proj(0, 1, x_c01)
    kv02 = kv_proj(0, 2, x_c02)

    # ---- Phase A: batched Q projection ----
    for i in range(NPASS_Q):
        dma_w(wqt[i], env["wqt_d"][i])
    x_items = pA.tile([128, CT, SAMP], F32R)
    for s in range(BPC):
        for t in range(CT):
            nc.sync.dma_start(
                x_items[:, t, s * HW:(s + 1) * HW],
                x_d[s, t * 128:(t + 1) * 128, ITEM0:ITEM0 + HW].bitcast(F32R))
    q_all = persist.tile([128, MI, SAMP], F32R)
    y_all = persist.tile([128, MI, SAMP], F32R)
    for m in range(MI):
        for (n0, nsz) in NCH:
            pq = pY_pool.tile([128, 512], F32, tag="py", name=f"pq{m}_{n0}")
            for ip in range(NPASS_Q):
                for t in range(CT):
                    nc.tensor.matmul(
                        pq[:, :nsz],
                        wqt[ip][:, t, m * 128:(m + 1) * 128],
                        x_items[:, t, n0:n0 + nsz],
                        start=(t == 0 and ip == 0),
                        stop=(t == CT - 1 and ip == NPASS_Q - 1))
            nc.scalar.activation(
                q_all[:, m, n0:n0 + nsz], pq[:, :nsz],
                mybir.ActivationFunctionType.Identity,
                bias=bq_sb[:, m:m + 1])
    pA.release()

    wot_pool = tc.alloc_tile_pool(name="wotp", bufs=1)
    wot = [wot_pool.tile([128, MI, C], F32R, tag=f"wot{i}", name=f"wot{i}")
           for i in range(NPASS_O)]
    for i in range(NPASS_O):
        dma_w(wot[i], env["wot_d"][i])
    late["e"] = tc.alloc_tile_pool(name="esb", bufs=3)
    late["et"] = tc.alloc_tile_pool(name="etsb", bufs=5)
    yt_pool = tc.alloc_tile_pool(name="ytsb", bufs=2)
    z_pool = tc.alloc_tile_pool(name="zsb", bufs=2)

    # ---- Phase B: per-sample attention ----
    n_kt_total = sum(len(_subtiles(csz)) for _, csz in CHUNKS)
    for s in range(BPC):
        py = [pY_pool.tile([128, CI], F32, tag="py", name=f"py{s}_{qi}")
              for qi in range(len(QT))]
        zs = [z_pool.tile([128, len(CHUNKS)], F32, tag="z", name=f"zs{s}_{qi}")
              for qi in range(len(QT))]
        kt_base = 0
        for ci in range(len(CHUNKS)):
            if first and s == 0 and ci == 0:
                k_sb, vts = kv00
            elif first and s == 0 and ci == 1:
                k_sb, vts = kv01
            elif first and s == 0 and ci == 2:
                k_sb, vts = kv02
            else:
                x_c = dma_x_chunk(s, ci)
                k_sb, vts = kv_proj(s, ci, x_c)
            attn_chunk(s, ci, k_sb, vts, q_all, py, zs, kt_base, n_kt_total)
            kt_base += len(_subtiles(CHUNKS[ci][1]))

        for qi, (qo, qsz) in enumerate(QT):
            zsum = z_pool.tile([128, 1], F32, tag="zsum", name=f"zsum{s}_{qi}")
            nc.vector.tensor_reduce(
                zsum[:qsz, :], zs[qi][:qsz, :],
                axis=mybir.AxisListType.X, op=mybir.AluOpType.add)
            rz = z_pool.tile([128, 1], F32, tag="rz", name=f"rz{s}_{qi}")
            nc.vector.reciprocal(rz[:qsz, :], zsum[:qsz, :])
            yt = yt_pool.tile([128, CI], F32, tag="yt", name=f"yt{s}_{qi}")
            nc.scalar.activation(
                yt[:qsz, :], py[qi][:qsz, :],
                mybir.ActivationFunctionType.Copy, scale=rz[:qsz, :])
            for m in range(MI):
                pyt = pT_pool.tile([128, 128], F32, tag="pt",
                                   name=f"pyt{s}_{qi}_{m}")
                nc.tensor.transpose(
                    pyt[:, :qsz], yt[:qsz, m * 128:(m + 1) * 128], ident[:qsz, :qsz])
                nc.scalar.activation(
                    y_all[:, m, s * HW + qo: s * HW + qo + qsz],
                    pyt[:, :qsz],
                    mybir.ActivationFunctionType.Identity,
                    bias=bv_sb[:, m:m + 1])

    for p in (pY_pool, pT_pool, pS_pool, pacc):
        p.release()

    # ---- Phase C: batched output projection ----
    out_pool = tc.alloc_tile_pool(name="outsb", bufs=3)
    psC = tc.alloc_tile_pool(name="psC", bufs=4, space="PSUM")
    for mo in range(CT):
        out_t = out_pool.tile([128, SAMP], F32, tag="out", name=f"out{mo}")
        for (n0, nsz) in NCH:
            po = psC.tile([128, 512], F32, tag="po", name=f"po{mo}_{n0}")
            for ip in range(NPASS_O):
                for m in range(MI):
                    nc.tensor.matmul(
                        po[:, :nsz],
                        wot[ip][:, m, mo * 128:(mo + 1) * 128],
                        y_all[:, m, n0:n0 + nsz],
                        start=(m == 0 and ip == 0),
                        stop=(m == MI - 1 and ip == NPASS_O - 1))
            nc.scalar.activation(
                out_t[:, n0:n0 + nsz], po[:, :nsz],
                mybir.ActivationFunctionType.Identity,
                bias=bo_sb[:, mo:mo + 1])
        nc.sync.dma_start(
            out_d[:, mo * 128:(mo + 1) * 128, :].rearrange("s p q -> p s q"),
            out_t[:].rearrange("p (s q) -> p s q", s=BPC))
    psC.release()
    out_pool.release()
    for p in (z_pool, yt_pool, late["et"], late["e"], wot_pool,
              vt_pool, k_pool, xc_pool, persist):
        p.release()


def _tf32_round(x):
    xi = x.view(np.uint32)
    r = ((xi.astype(np.uint64) + 0x1000) & 0xFFFFE000).astype(np.uint32)
    return r.view(np.float32)


def _split_passes(wt, npass):
    """wt: already-transposed weight (contraction-major). Returns npass arrays."""
    wt = np.ascontiguousarray(wt, dtype=np.float32)
    if npass == 1:
        return [wt]
    hi = _tf32_round(wt)
    lo = (wt - hi).astype(np.float32)
    return [hi, lo]


_NC_CACHE = {}


def _get_nc():
    key = (NPASS_Q, NPASS_K, NPASS_V, NPASS_O, REPEAT)
    if key not in _NC_CACHE:
        _NC_CACHE[key] = build_kernel()
    return _NC_CACHE[key]


def kernel(x_window, Wq, bq, Wk, bk, Wv, bv, Wo, bo):
    nc = _get_nc()

    x_window = np.ascontiguousarray(x_window, dtype=np.float32)
    x_flat = x_window.reshape(B, C, THW)

    wqt = _split_passes(np.asarray(Wq, np.float32).T, NPASS_Q)   # (C, CI)
    wkt = _split_passes(np.asarray(Wk, np.float32).T, NPASS_K)
    wvt = _split_passes(np.asarray(Wv, np.float32).T, NPASS_V)
    wot = _split_passes(np.asarray(Wo, np.float32).T, NPASS_O)   # (CI, C)

    shared = {}
    for pre, ws in (("wqt", wqt), ("wkt", wkt), ("wvt", wvt), ("wot", wot)):
        for i, w in enumerate(ws):
            shared[f"{pre}{i}"] = w
    shared["bq"] = np.ascontiguousarray(np.asarray(bq, np.float32).reshape(MI, 128))
    shared["bk"] = np.ascontiguousarray(np.asarray(bk, np.float32).reshape(MI, 128))
    shared["bv"] = np.ascontiguousarray(np.asarray(bv, np.float32).reshape(MI, 128))
    shared["bo"] = np.ascontiguousarray(np.asarray(bo, np.float32).reshape(CT, 128))

    in_maps = []
    for i in range(NCORES):
        m = dict(shared)
        m["x"] = np.ascontiguousarray(x_flat[i * BPC:(i + 1) * BPC])
        in_maps.append(m)

    res = run_bass_kernel_spmd(nc, in_maps, list(range(NCORES)))
    out = np.concatenate([res.results[i]["out"] for i in range(NCORES)], axis=0)
    return out.reshape(B, C, 1, H, W)



# revision 3
# speedup vs baseline: 4.6504x; 4.6504x over previous
"""Trainium2 Bass kernel for LocalSelfAttentionHeadSum.

Reference computation (per sample b of B=32):
  x = x_window[b] reshaped (C=1024, THW=1764); x_item = cols 784:980 (center)
  q = Wq @ x_item + bq          (512, 196)
  k = Wk @ x + bk               (512, 1764)
  v = Wv @ x + bv               (512, 1764)
  alpha = softmax(q^T k, axis=-1)
  y = v @ alpha^T               (512, 196)
  out = Wo @ y + bo             (1024, 196)

Sharding: data-parallel over B across 8 cores (4 samples per core).

Precision scheme: fp16 operands everywhere (10-bit mantissa == tf32, so a
single fp16 pass matches the old single-pass fp32r accuracy at full PE rate
and at any moving-dim size), EXCEPT the exp(S) values and v^T which are bf16
(exp(S) reaches ~e^60 ~ 1e26, far beyond fp16 range; bf16 has fp32-range
exponent).  All matmul accumulation is fp32 in PSUM.  No max-subtraction in
softmax: |S| <~ 60 stays inside bf16/fp32 exp range.

Layout: scores are computed directly transposed (S^T = k_sb^T q, keys on
partitions) which kills the per-chunk PE transposes of exp(S); the AV matmul
then produces y^T with a ones-column appended to v^T so the softmax
denominator Z accumulates as column 512 of the same PSUM tile.  y^T is
normalized by 1/Z (per-partition scale), PE-transposed to y, biased with bv
(valid since sum(alpha)=1), and output-projected.

Pipelining: per-sample emission order is proj(s+1) [K,V,Q] -> tail(s)
[normalize, y-transpose, out-proj, DMA out] -> attn(s+1), so the PE fills the
normalize/O latency of sample s with sample s+1's projection work.  PSUM is
exactly 8 banks: 2 rotating accumulators (K/V/Q/O), 2 rotating S^T/transpose
banks, 4 attention-output banks (2 query-tiles x 2 halves, Z in the second
half's col 256... col 512 overall).
"""

import os
import numpy as np

import concourse.bass as bass
import concourse.tile as tile
from concourse import bacc, mybir
from concourse.bass_utils import run_bass_kernel_spmd
from concourse.masks import make_identity

F32 = mybir.dt.float32
F16 = mybir.dt.float16
BF16 = mybir.dt.bfloat16

# Problem shapes (hardcoded per contract)
B, C, T, H, W = 32, 1024, 9, 14, 14
CI = 512
HW = H * W              # 196
THW = T * HW            # 1764
NCORES = 8
BPC = B // NCORES       # 4 samples per core
CT = C // 128           # 8 C-tiles
MI = CI // 128          # 4 Ci-tiles
ITEM0 = (T // 2) * HW   # 784, center-frame column offset
QT = [(0, 128), (128, HW - 128)]                  # query partition tiles
NJ = 4
JW = THW // NJ                                    # 441, K-proj PSUM slice
KT = [(o, min(128, THW - o)) for o in range(0, THW, 128)]  # 14 key subtiles
NKT = len(KT)

REPEAT = int(os.environ.get("KREPEAT", "1"))
IDENT = mybir.ActivationFunctionType.Identity
COPY = mybir.ActivationFunctionType.Copy
EXP = mybir.ActivationFunctionType.Exp


def build_kernel(repeat):
    nc = bacc.Bacc("TRN2", target_bir_lowering=False, debug=False)

    x_d = nc.dram_tensor("x", [BPC, C, THW], F16, kind="ExternalInput")
    wqt_d = nc.dram_tensor("wqt", [C, CI], F16, kind="ExternalInput")
    wkt_d = nc.dram_tensor("wkt", [C, CI], F16, kind="ExternalInput")
    wvt_d = nc.dram_tensor("wvt", [C, CI], F16, kind="ExternalInput")
    wot_d = nc.dram_tensor("wot", [CI, C], F16, kind="ExternalInput")
    bq_d = nc.dram_tensor("bq", [MI, 128], F32, kind="ExternalInput")
    bk_d = nc.dram_tensor("bk", [MI, 128], F32, kind="ExternalInput")
    bv_d = nc.dram_tensor("bv", [MI, 128], F32, kind="ExternalInput")
    bo_d = nc.dram_tensor("bo", [CT, 128], F32, kind="ExternalInput")
    out_d = nc.dram_tensor("out", [BPC, C, HW], F32, kind="ExternalOutput")

    with tile.TileContext(nc) as tc:
        with tc.tile_pool(name="const", bufs=1) as const_pool:
            ident = const_pool.tile([128, 128], F16)
            bq_sb = const_pool.tile([128, MI], F32)
            bk_sb = const_pool.tile([128, MI], F32)
            bv_sb = const_pool.tile([128, MI], F32)
            bo_sb = const_pool.tile([128, CT], F32)

            env = dict(
                nc=nc, tc=tc, x_d=x_d, out_d=out_d,
                wqt_d=wqt_d, wkt_d=wkt_d, wvt_d=wvt_d, wot_d=wot_d,
                bq_d=bq_d, bk_d=bk_d, bv_d=bv_d, bo_d=bo_d,
                ident=ident, bq_sb=bq_sb, bk_sb=bk_sb, bv_sb=bv_sb,
                bo_sb=bo_sb,
            )
            for rep in range(repeat):
                _emit_iteration(env, first=(rep == 0))

    nc.compile()
    return nc


def _emit_iteration(env, first):
    nc, tc = env["nc"], env["tc"]
    x_d, out_d = env["x_d"], env["out_d"]
    ident = env["ident"]
    bq_sb, bk_sb, bv_sb, bo_sb = (env["bq_sb"], env["bk_sb"],
                                  env["bv_sb"], env["bo_sb"])

    w_pool = tc.alloc_tile_pool(name="wp", bufs=1)
    x_pool = tc.alloc_tile_pool(name="xp", bufs=2)
    k_pool = tc.alloc_tile_pool(name="kp", bufs=2)
    q_pool = tc.alloc_tile_pool(name="qp", bufs=2)
    v_pool = tc.alloc_tile_pool(name="vp", bufs=30)
    et_pool = tc.alloc_tile_pool(name="etp", bufs=3)
    yt_pool = tc.alloc_tile_pool(name="ytp", bufs=2)
    ys_pool = tc.alloc_tile_pool(name="ysp", bufs=2)
    o_pool = tc.alloc_tile_pool(name="osb", bufs=3)
    z_pool = tc.alloc_tile_pool(name="zp", bufs=4)
    pacc = tc.alloc_tile_pool(name="pacc", bufs=2, space="PSUM")
    pS = tc.alloc_tile_pool(name="pS", bufs=2, space="PSUM")
    pY = tc.alloc_tile_pool(name="pY", bufs=2, space="PSUM")

    # ---- weights + first x: DMA order is the startup lever ----
    wkt = w_pool.tile([128, CT, CI], F16, tag="wkt", name="wkt")
    wvt = w_pool.tile([128, CT, CI], F16, tag="wvt", name="wvt")
    wqt = w_pool.tile([128, CT, CI], F16, tag="wqt", name="wqt")
    wot = w_pool.tile([128, MI, C], F16, tag="wot", name="wot")
    for t in range(CT):
        nc.sync.dma_start(wkt[:, t, :], env["wkt_d"][t * 128:(t + 1) * 128, :])

    def dma_x(s):
        x_s = x_pool.tile([128, CT, THW], F16, tag="x", name=f"x{s}")
        for t in range(CT):
            nc.sync.dma_start(
                x_s[:, t, :], x_d[s, t * 128:(t + 1) * 128, :])
        return x_s

    xs = {0: dma_x(0)}
    for t in range(CT):
        nc.sync.dma_start(wvt[:, t, :], env["wvt_d"][t * 128:(t + 1) * 128, :])
    for t in range(CT):
        nc.sync.dma_start(wqt[:, t, :], env["wqt_d"][t * 128:(t + 1) * 128, :])
    for m in range(MI):
        nc.sync.dma_start(wot[:, m, :], env["wot_d"][m * 128:(m + 1) * 128, :])
    if first:
        nc.sync.dma_start(bq_sb[:], env["bq_d"][:].rearrange("m p -> p m"))
        nc.sync.dma_start(bk_sb[:], env["bk_d"][:].rearrange("m p -> p m"))
        nc.sync.dma_start(bv_sb[:], env["bv_d"][:].rearrange("m p -> p m"))
        nc.sync.dma_start(bo_sb[:], env["bo_d"][:].rearrange("m p -> p m"))
        make_identity(nc, ident[:])

    state = {}

    def proj(s):
        """K, V, Q projections of sample s (PE + ACT/DVE epilogues)."""
        if s + 1 < BPC:
            xs[s + 1] = dma_x(s + 1)
        x_s = xs[s]
        k_sb = k_pool.tile([128, MI, THW], F16, tag="k", name=f"k{s}")
        for m in range(MI):
            for j in range(NJ):
                pk = pacc.tile([128, 512], F32, tag="acc", name=f"pk{s}_{m}_{j}")
                for t in range(CT):
                    nc.tensor.matmul(
                        pk[:, :JW],
                        wkt[:, t, m * 128:(m + 1) * 128],
                        x_s[:, t, j * JW:(j + 1) * JW],
                        start=(t == 0), stop=(t == CT - 1))
                nc.scalar.activation(
                    k_sb[:, m, j * JW:(j + 1) * JW], pk[:, :JW],
                    IDENT, bias=bk_sb[:, m:m + 1])
        vts = []
        for kj, (ko, ksz) in enumerate(KT):
            pv = pacc.tile([128, 512], F32, tag="acc", name=f"pv{s}_{kj}")
            for t in range(CT):
                nc.tensor.matmul(
                    pv[:ksz, :],
                    x_s[:, t, ko:ko + ksz],
                    wvt[:, t, :],
                    start=(t == 0), stop=(t == CT - 1))
            vt = v_pool.tile([128, CI + 1], BF16, tag="vt", name=f"vt{s}_{kj}")
            nc.vector.tensor_copy(vt[:ksz, :CI], pv[:ksz, :])
            nc.vector.memset(vt[:ksz, CI:CI + 1], 1.0)
            vts.append(vt)
        q_s = q_pool.tile([128, MI, HW], F16, tag="q", name=f"q{s}")
        for m in range(MI):
            pq = pacc.tile([128, 512], F32, tag="acc", name=f"pq{s}_{m}")
            for t in range(CT):
                nc.tensor.matmul(
                    pq[:, :HW],
                    wqt[:, t, m * 128:(m + 1) * 128],
                    x_s[:, t, ITEM0:ITEM0 + HW],
                    start=(t == 0), stop=(t == CT - 1))
            nc.scalar.activation(
                q_s[:, m, :], pq[:, :HW], IDENT, bias=bq_sb[:, m:m + 1])
        state[s] = dict(k_sb=k_sb, q_s=q_s, vts=vts)

    def attn(s):
        """S^T scores -> exp -> AV (y^T with Z in col 512 of half b)."""
        st = state[s]
        k_sb, q_s, vts = st["k_sb"], st["q_s"], st["vts"]
        py = {}
        for qi in range(len(QT)):
            py[qi, 0] = pY.tile([128, 256], F32, tag="pya", name=f"pya{s}_{qi}")
            py[qi, 1] = pY.tile([128, 257], F32, tag="pyb", name=f"pyb{s}_{qi}")
        ets = [None] * NKT

        def st_group(kj):
            ko, ksz = KT[kj]
            psT = pS.tile([128, 196], F32, tag="ps", name=f"ps{s}_{kj}")
            for m in range(MI):
                nc.tensor.matmul(
                    psT[:ksz, :],
                    k_sb[:, m, ko:ko + ksz],
                    q_s[:, m, :],
                    start=(m == 0), stop=(m == MI - 1))
            et = et_pool.tile([128, HW], BF16, tag="et", name=f"et{s}_{kj}")
            nc.scalar.activation(et[:ksz, :], psT[:ksz, :], EXP)
            ets[kj] = et

        def av_group(kj):
            ko, ksz = KT[kj]
            et = ets[kj]
            for qi, (qo, qsz) in enumerate(QT):
                nc.tensor.matmul(
                    py[qi, 0][:qsz, :],
                    et[:ksz, qo:qo + qsz],
                    vts[kj][:ksz, 0:256],
                    start=(kj == 0), stop=(kj == NKT - 1))
                nc.tensor.matmul(
                    py[qi, 1][:qsz, :],
                    et[:ksz, qo:qo + qsz],
                    vts[kj][:ksz, 256:CI + 1],
                    start=(kj == 0), stop=(kj == NKT - 1))

        for kj in range(NKT):
            st_group(kj)
            if kj >= 1:
                av_group(kj - 1)
        av_group(NKT - 1)
        st["py"] = py

    def tail(s):
        """normalize y^T by 1/Z, transpose to y, add bv, out-proj, DMA."""
        py = state[s]["py"]
        y_s = ys_pool.tile([128, MI, HW], F16, tag="y", name=f"y{s}")
        for qi, (qo, qsz) in enumerate(QT):
            rz = z_pool.tile([128, 1], F32, tag="rz", name=f"rz{s}_{qi}")
            nc.vector.reciprocal(rz[:qsz, :], py[qi, 1][:qsz, 256:257])
            yt = yt_pool.tile([128, CI], F16, tag="yt", name=f"yt{s}_{qi}")
            nc.scalar.activation(
                yt[:qsz, 0:256], py[qi, 0][:qsz, :], COPY, scale=rz[:qsz, :])
            nc.scalar.activation(
                yt[:qsz, 256:CI], py[qi, 1][:qsz, 0:256], COPY,
                scale=rz[:qsz, :])
            for m in range(MI):
                pT = pS.tile([128, 196], F16, tag="ps", name=f"pt{s}_{qi}_{m}")
                nc.tensor.transpose(
                    pT[:, :qsz], yt[:qsz, m * 128:(m + 1) * 128],
                    ident[:qsz, :qsz])
                nc.scalar.activation(
                    y_s[:, m, qo:qo + qsz], pT[:, :qsz], IDENT,
                    bias=bv_sb[:, m:m + 1])
        for mo in range(CT):
            po = pacc.tile([128, 512], F32, tag="acc", name=f"po{s}_{mo}")
            for m in range(MI):
                nc.tensor.matmul(
                    po[:, :HW],
                    wot[:, m, mo * 128:(mo + 1) * 128],
                    y_s[:, m, :],
                    start=(m == 0), stop=(m == MI - 1))
            out_t = o_pool.tile([128, HW], F32, tag="o", name=f"o{s}_{mo}")
            nc.scalar.activation(
                out_t[:], po[:, :HW], IDENT, bias=bo_sb[:, mo:mo + 1])
            nc.sync.dma_start(out_d[s, mo * 128:(mo + 1) * 128, :], out_t[:])
        del state[s]

    # proj(s+1) sits between attn(s) and tail(s) so the PE fills the
    # normalize latency of sample s with sample s+1's projections.
    proj(0)
    attn(0)
    for s in range(1, BPC):
        proj(s)
        tail(s - 1)
        attn(s)
    tail(BPC - 1)

    for p in (pY, pS, pacc, z_pool, o_pool, ys_pool, yt_pool, et_pool,
              v_pool, q_pool, k_pool, x_pool, w_pool):
        p.release()


_NC_CACHE = {}


def _get_nc():
    key = REPEAT
    if key not in _NC_CACHE:
        _NC_CACHE[key] = build_kernel(REPEAT)
    return _NC_CACHE[key]


def _make_in_maps(inputs):
    x_flat = np.asarray(inputs["x_window"], np.float32).reshape(B, C, THW)
    x16 = x_flat.astype(np.float16)
    shared = {
        "wqt": np.ascontiguousarray(np.asarray(inputs["Wq"], np.float32).T
                                    .astype(np.float16)),
        "wkt": np.ascontiguousarray(np.asarray(inputs["Wk"], np.float32).T
                                    .astype(np.float16)),
        "wvt": np.ascontiguousarray(np.asarray(inputs["Wv"], np.float32).T
                                    .astype(np.float16)),
        "wot": np.ascontiguousarray(np.asarray(inputs["Wo"], np.float32).T
                                    .astype(np.float16)),
        "bq": np.ascontiguousarray(
            np.asarray(inputs["bq"], np.float32).reshape(MI, 128)),
        "bk": np.ascontiguousarray(
            np.asarray(inputs["bk"], np.float32).reshape(MI, 128)),
        "bv": np.ascontiguousarray(
            np.asarray(inputs["bv"], np.float32).reshape(MI, 128)),
        "bo": np.ascontiguousarray(
            np.asarray(inputs["bo"], np.float32).reshape(CT, 128)),
    }
    in_maps = []
    for i in range(NCORES):
        m = dict(shared)
        m["x"] = np.ascontiguousarray(x16[i * BPC:(i + 1) * BPC])
        in_maps.append(m)
    return in_maps


def kernel(x_window, Wq, bq, Wk, bk, Wv, bv, Wo, bo):
    nc = _get_nc()
    in_maps = _make_in_maps(dict(
        x_window=x_window, Wq=Wq, bq=bq, Wk=Wk, bk=bk, Wv=Wv, bv=bv,
        Wo=Wo, bo=bo))
    res = run_bass_kernel_spmd(nc, in_maps, list(range(NCORES)))
    out = np.concatenate([res.results[i]["out"] for i in range(NCORES)],
                         axis=0)
    return out.reshape(B, C, 1, H, W)


# revision 4
# speedup vs baseline: 96892.1066x; 20835.4413x over previous
"""Trainium2 Bass kernel for LocalSelfAttentionHeadSum.

Reference computation (per sample b of B=32):
  x = x_window[b] reshaped (C=1024, THW=1764); x_item = cols 784:980 (center)
  q = Wq @ x_item + bq          (512, 196)
  k = Wk @ x + bk               (512, 1764)
  v = Wv @ x + bv               (512, 1764)
  alpha = softmax(q^T k, axis=-1)
  y = v @ alpha^T               (512, 196)
  out = Wo @ y + bo             (1024, 196)

Sharding: data-parallel over B across 8 cores (4 samples per core).

Precision scheme: fp16 operands everywhere (10-bit mantissa == tf32, so a
single fp16 pass matches the old single-pass fp32r accuracy at full PE rate
and at any moving-dim size), EXCEPT the exp(S) values and v^T which are bf16
(exp(S) reaches ~e^60 ~ 1e26, far beyond fp16 range; bf16 has fp32-range
exponent).  All matmul accumulation is fp32 in PSUM.  No max-subtraction in
softmax: |S| <~ 60 stays inside bf16/fp32 exp range.

Layout: scores are computed directly transposed (S^T = k_sb^T q, keys on
partitions) which kills the per-chunk PE transposes of exp(S); the AV matmul
then produces y^T with a ones-column appended to v^T so the softmax
denominator Z accumulates as column 512 of the same PSUM tile.  y^T is
normalized by 1/Z (per-partition scale), PE-transposed to y, biased with bv
(valid since sum(alpha)=1), and output-projected.

Pipelining: per-sample emission order is proj(s+1) [K,V,Q] -> tail(s)
[normalize, y-transpose, out-proj, DMA out] -> attn(s+1), so the PE fills the
normalize/O latency of sample s with sample s+1's projection work.  PSUM is
exactly 8 banks: 2 rotating accumulators (K/V/Q/O), 2 rotating S^T/transpose
banks, 4 attention-output banks (2 query-tiles x 2 halves, Z in the second
half's col 256... col 512 overall).
"""

import os
import numpy as np

import concourse.bass as bass
import concourse.tile as tile
from concourse import bacc, mybir
from concourse.bass_utils import run_bass_kernel_spmd
from concourse.masks import make_identity

F32 = mybir.dt.float32
F16 = mybir.dt.float16
BF16 = mybir.dt.bfloat16

# Problem shapes (hardcoded per contract)
B, C, T, H, W = 32, 1024, 9, 14, 14
CI = 512
HW = H * W              # 196
THW = T * HW            # 1764
NCORES = 8
BPC = B // NCORES       # 4 samples per core
CT = C // 128           # 8 C-tiles
MI = CI // 128          # 4 Ci-tiles
ITEM0 = (T // 2) * HW   # 784, center-frame column offset
QT = [(0, 128), (128, HW - 128)]                  # query partition tiles
NJ = 4
JW = THW // NJ                                    # 441, K-proj PSUM slice
KT = [(o, min(128, THW - o)) for o in range(0, THW, 128)]  # 14 key subtiles
NKT = len(KT)

REPEAT = int(os.environ.get("KREPEAT", "1"))
IDENT = mybir.ActivationFunctionType.Identity
COPY = mybir.ActivationFunctionType.Copy
EXP = mybir.ActivationFunctionType.Exp


def build_kernel(repeat):
    nc = bacc.Bacc("TRN2", target_bir_lowering=False, debug=False)

    x_d = nc.dram_tensor("x", [BPC, C, THW], F16, kind="ExternalInput")
    wqt_d = nc.dram_tensor("wqt", [C, CI], F16, kind="ExternalInput")
    wkt_d = nc.dram_tensor("wkt", [C, CI], F16, kind="ExternalInput")
    wvt_d = nc.dram_tensor("wvt", [C, CI], F16, kind="ExternalInput")
    wot_d = nc.dram_tensor("wot", [CI, C], F16, kind="ExternalInput")
    bq_d = nc.dram_tensor("bq", [MI, 128], F32, kind="ExternalInput")
    bk_d = nc.dram_tensor("bk", [MI, 128], F32, kind="ExternalInput")
    bv_d = nc.dram_tensor("bv", [MI, 128], F32, kind="ExternalInput")
    bo_d = nc.dram_tensor("bo", [CT, 128], F32, kind="ExternalInput")
    out_d = nc.dram_tensor("out", [BPC, C, HW], F32, kind="ExternalOutput")

    with tile.TileContext(nc) as tc:
        with tc.tile_pool(name="const", bufs=1) as const_pool:
            ident = const_pool.tile([128, 128], F16)
            bq_sb = const_pool.tile([128, MI], F32)
            bk_sb = const_pool.tile([128, MI], F32)
            bv_sb = const_pool.tile([128, MI], F32)
            bo_sb = const_pool.tile([128, CT], F32)

            env = dict(
                nc=nc, tc=tc, x_d=x_d, out_d=out_d,
                wqt_d=wqt_d, wkt_d=wkt_d, wvt_d=wvt_d, wot_d=wot_d,
                bq_d=bq_d, bk_d=bk_d, bv_d=bv_d, bo_d=bo_d,
                ident=ident, bq_sb=bq_sb, bk_sb=bk_sb, bv_sb=bv_sb,
                bo_sb=bo_sb,
            )
            for rep in range(repeat):
                _emit_iteration(env, first=(rep == 0))

    nc.compile()
    return nc


def _emit_iteration(env, first):
    nc, tc = env["nc"], env["tc"]
    x_d, out_d = env["x_d"], env["out_d"]
    ident = env["ident"]
    bq_sb, bk_sb, bv_sb, bo_sb = (env["bq_sb"], env["bk_sb"],
                                  env["bv_sb"], env["bo_sb"])

    w_pool = tc.alloc_tile_pool(name="wp", bufs=1)
    x_pool = tc.alloc_tile_pool(name="xp", bufs=2)
    k_pool = tc.alloc_tile_pool(name="kp", bufs=2)
    q_pool = tc.alloc_tile_pool(name="qp", bufs=2)
    v_pool = tc.alloc_tile_pool(name="vp", bufs=30)
    et_pool = tc.alloc_tile_pool(name="etp", bufs=3)
    yt_pool = tc.alloc_tile_pool(name="ytp", bufs=2)
    ys_pool = tc.alloc_tile_pool(name="ysp", bufs=2)
    o_pool = tc.alloc_tile_pool(name="osb", bufs=3)
    z_pool = tc.alloc_tile_pool(name="zp", bufs=4)
    pacc = tc.alloc_tile_pool(name="pacc", bufs=2, space="PSUM")
    pS = tc.alloc_tile_pool(name="pS", bufs=2, space="PSUM")
    pY = tc.alloc_tile_pool(name="pY", bufs=2, space="PSUM")

    # ---- weights + first x: DMA order is the startup lever ----
    wkt = w_pool.tile([128, CT, CI], F16, tag="wkt", name="wkt")
    wvt = w_pool.tile([128, CT, CI], F16, tag="wvt", name="wvt")
    wqt = w_pool.tile([128, CT, CI], F16, tag="wqt", name="wqt")
    wot = w_pool.tile([128, MI, C], F16, tag="wot", name="wot")
    for t in range(CT):
        nc.scalar.dma_start(wkt[:, t, :], env["wkt_d"][t * 128:(t + 1) * 128, :])

    def dma_x(s):
        x_s = x_pool.tile([128, CT, THW], F16, tag="x", name=f"x{s}")
        for t in range(CT):
            nc.sync.dma_start(
                x_s[:, t, :], x_d[s, t * 128:(t + 1) * 128, :])
        return x_s

    xs = {0: dma_x(0)}
    for t in range(CT):
        nc.scalar.dma_start(wvt[:, t, :], env["wvt_d"][t * 128:(t + 1) * 128, :])
    for t in range(CT):
        nc.scalar.dma_start(wqt[:, t, :], env["wqt_d"][t * 128:(t + 1) * 128, :])
    for m in range(MI):
        nc.scalar.dma_start(wot[:, m, :], env["wot_d"][m * 128:(m + 1) * 128, :])
    if first:
        nc.scalar.dma_start(bq_sb[:], env["bq_d"][:].rearrange("m p -> p m"))
        nc.scalar.dma_start(bk_sb[:], env["bk_d"][:].rearrange("m p -> p m"))
        nc.scalar.dma_start(bv_sb[:], env["bv_d"][:].rearrange("m p -> p m"))
        nc.scalar.dma_start(bo_sb[:], env["bo_d"][:].rearrange("m p -> p m"))
        make_identity(nc, ident[:])

    state = {}

    def proj(s):
        """K, V, Q projections of sample s (PE + ACT/DVE epilogues)."""
        if s + 1 < BPC:
            xs[s + 1] = dma_x(s + 1)
        x_s = xs[s]
        k_sb = k_pool.tile([128, MI, THW], F16, tag="k", name=f"k{s}")
        for m in range(MI):
            for j in range(NJ):
                pk = pacc.tile([128, 512], F32, tag="acc", name=f"pk{s}_{m}_{j}")
                for t in range(CT):
                    nc.tensor.matmul(
                        pk[:, :JW],
                        wkt[:, t, m * 128:(m + 1) * 128],
                        x_s[:, t, j * JW:(j + 1) * JW],
                        start=(t == 0), stop=(t == CT - 1))
                nc.scalar.activation(
                    k_sb[:, m, j * JW:(j + 1) * JW], pk[:, :JW],
                    IDENT, bias=bk_sb[:, m:m + 1])
        vts = []
        for kj, (ko, ksz) in enumerate(KT):
            pv = pacc.tile([128, 512], F32, tag="acc", name=f"pv{s}_{kj}")
            for t in range(CT):
                nc.tensor.matmul(
                    pv[:ksz, :],
                    x_s[:, t, ko:ko + ksz],
                    wvt[:, t, :],
                    start=(t == 0), stop=(t == CT - 1))
            vt = v_pool.tile([128, CI + 1], BF16, tag="vt", name=f"vt{s}_{kj}")
            nc.vector.tensor_copy(vt[:ksz, :CI], pv[:ksz, :])
            nc.vector.memset(vt[:ksz, CI:CI + 1], 1.0)
            vts.append(vt)
        q_s = q_pool.tile([128, MI, HW], F16, tag="q", name=f"q{s}")
        for m in range(MI):
            pq = pacc.tile([128, 512], F32, tag="acc", name=f"pq{s}_{m}")
            for t in range(CT):
                nc.tensor.matmul(
                    pq[:, :HW],
                    wqt[:, t, m * 128:(m + 1) * 128],
                    x_s[:, t, ITEM0:ITEM0 + HW],
                    start=(t == 0), stop=(t == CT - 1))
            nc.scalar.activation(
                q_s[:, m, :], pq[:, :HW], IDENT, bias=bq_sb[:, m:m + 1])
        state[s] = dict(k_sb=k_sb, q_s=q_s, vts=vts)

    def attn(s):
        """S^T scores -> exp -> AV (y^T with Z in col 512 of half b)."""
        st = state[s]
        k_sb, q_s, vts = st["k_sb"], st["q_s"], st["vts"]
        py = {}
        for qi in range(len(QT)):
            py[qi, 0] = pY.tile([128, 256], F32, tag="pya", name=f"pya{s}_{qi}")
            py[qi, 1] = pY.tile([128, 257], F32, tag="pyb", name=f"pyb{s}_{qi}")
        ets = [None] * NKT

        def st_group(kj):
            ko, ksz = KT[kj]
            psT = pS.tile([128, 196], F32, tag="ps", name=f"ps{s}_{kj}")
            for m in range(MI):
                nc.tensor.matmul(
                    psT[:ksz, :],
                    k_sb[:, m, ko:ko + ksz],
                    q_s[:, m, :],
                    start=(m == 0), stop=(m == MI - 1))
            et = et_pool.tile([128, HW], BF16, tag="et", name=f"et{s}_{kj}")
            nc.scalar.activation(et[:ksz, :], psT[:ksz, :], EXP)
            ets[kj] = et

        def av_group(kj):
            ko, ksz = KT[kj]
            et = ets[kj]
            for qi, (qo, qsz) in enumerate(QT):
                nc.tensor.matmul(
                    py[qi, 0][:qsz, :],
                    et[:ksz, qo:qo + qsz],
                    vts[kj][:ksz, 0:256],
                    start=(kj == 0), stop=(kj == NKT - 1))
                nc.tensor.matmul(
                    py[qi, 1][:qsz, :],
                    et[:ksz, qo:qo + qsz],
                    vts[kj][:ksz, 256:CI + 1],
                    start=(kj == 0), stop=(kj == NKT - 1))

        for kj in range(NKT):
            st_group(kj)
            if kj >= 1:
                av_group(kj - 1)
        av_group(NKT - 1)
        st["py"] = py

    def tail(s):
        """normalize y^T by 1/Z, transpose to y, add bv, out-proj, DMA."""
        py = state[s]["py"]
        y_s = ys_pool.tile([128, MI, HW], F16, tag="y", name=f"y{s}")
        for qi, (qo, qsz) in enumerate(QT):
            rz = z_pool.tile([128, 1], F32, tag="rz", name=f"rz{s}_{qi}")
            nc.vector.reciprocal(rz[:qsz, :], py[qi, 1][:qsz, 256:257])
            yt = yt_pool.tile([128, CI], F16, tag="yt", name=f"yt{s}_{qi}")
            nc.scalar.activation(
                yt[:qsz, 0:256], py[qi, 0][:qsz, :], COPY, scale=rz[:qsz, :])
            nc.scalar.activation(
                yt[:qsz, 256:CI], py[qi, 1][:qsz, 0:256], COPY,
                scale=rz[:qsz, :])
            for m in range(MI):
                pT = pS.tile([128, 196], F16, tag="ps", name=f"pt{s}_{qi}_{m}")
                nc.tensor.transpose(
                    pT[:, :qsz], yt[:qsz, m * 128:(m + 1) * 128],
                    ident[:qsz, :qsz])
                nc.scalar.activation(
                    y_s[:, m, qo:qo + qsz], pT[:, :qsz], IDENT,
                    bias=bv_sb[:, m:m + 1])
        for mo in range(CT):
            po = pacc.tile([128, 512], F32, tag="acc", name=f"po{s}_{mo}")
            for m in range(MI):
                nc.tensor.matmul(
                    po[:, :HW],
                    wot[:, m, mo * 128:(mo + 1) * 128],
                    y_s[:, m, :],
                    start=(m == 0), stop=(m == MI - 1))
            out_t = o_pool.tile([128, HW], F32, tag="o", name=f"o{s}_{mo}")
            nc.scalar.activation(
                out_t[:], po[:, :HW], IDENT, bias=bo_sb[:, mo:mo + 1])
            nc.scalar.dma_start(out_d[s, mo * 128:(mo + 1) * 128, :], out_t[:])
        del state[s]

    # proj(s+1) sits between attn(s) and tail(s) so the PE fills the
    # normalize latency of sample s with sample s+1's projections.
    proj(0)
    attn(0)
    for s in range(1, BPC):
        proj(s)
        tail(s - 1)
        attn(s)
    tail(BPC - 1)

    for p in (pY, pS, pacc, z_pool, o_pool, ys_pool, yt_pool, et_pool,
              v_pool, q_pool, k_pool, x_pool, w_pool):
        p.release()


_NC_CACHE = {}


def _get_nc():
    key = REPEAT
    if key not in _NC_CACHE:
        _NC_CACHE[key] = build_kernel(REPEAT)
    return _NC_CACHE[key]


def _make_in_maps(inputs):
    x_flat = np.asarray(inputs["x_window"], np.float32).reshape(B, C, THW)
    x16 = x_flat.astype(np.float16)
    shared = {
        "wqt": np.ascontiguousarray(np.asarray(inputs["Wq"], np.float32).T
                                    .astype(np.float16)),
        "wkt": np.ascontiguousarray(np.asarray(inputs["Wk"], np.float32).T
                                    .astype(np.float16)),
        "wvt": np.ascontiguousarray(np.asarray(inputs["Wv"], np.float32).T
                                    .astype(np.float16)),
        "wot": np.ascontiguousarray(np.asarray(inputs["Wo"], np.float32).T
                                    .astype(np.float16)),
        "bq": np.ascontiguousarray(
            np.asarray(inputs["bq"], np.float32).reshape(MI, 128)),
        "bk": np.ascontiguousarray(
            np.asarray(inputs["bk"], np.float32).reshape(MI, 128)),
        "bv": np.ascontiguousarray(
            np.asarray(inputs["bv"], np.float32).reshape(MI, 128)),
        "bo": np.ascontiguousarray(
            np.asarray(inputs["bo"], np.float32).reshape(CT, 128)),
    }
    in_maps = []
    for i in range(NCORES):
        m = dict(shared)
        m["x"] = np.ascontiguousarray(x16[i * BPC:(i + 1) * BPC])
        in_maps.append(m)
    return in_maps


def kernel(x_window, Wq, bq, Wk, bk, Wv, bv, Wo, bo):
    nc = _get_nc()
    in_maps = _make_in_maps(dict(
        x_window=x_window, Wq=Wq, bq=bq, Wk=Wk, bk=bk, Wv=Wv, bv=bv,
        Wo=Wo, bo=bo))
    res = run_bass_kernel_spmd(nc, in_maps, list(range(NCORES)))
    out = np.concatenate([res.results[i]["out"] for i in range(NCORES)],
                         axis=0)
    return out.reshape(B, C, 1, H, W)


# revision 5
# speedup vs baseline: 252440.7476x; 2.6054x over previous
"""Trainium2 Bass kernel for LocalSelfAttentionHeadSum.

Reference computation (per sample b of B=32):
  x = x_window[b] reshaped (C=1024, THW=1764); x_item = cols 784:980 (center)
  q = Wq @ x_item + bq          (512, 196)
  k = Wk @ x + bk               (512, 1764)
  v = Wv @ x + bv               (512, 1764)
  alpha = softmax(q^T k, axis=-1)
  y = v @ alpha^T               (512, 196)
  out = Wo @ y + bo             (1024, 196)

Sharding: data-parallel over B across 8 cores (4 samples per core).

Precision scheme: fp16 operands everywhere (10-bit mantissa == tf32, so a
single fp16 pass matches the old single-pass fp32r accuracy at full PE rate
and at any moving-dim size), EXCEPT the exp(S) values and v^T which are bf16
(exp(S) reaches ~e^60 ~ 1e26, far beyond fp16 range; bf16 has fp32-range
exponent).  All matmul accumulation is fp32 in PSUM.  No max-subtraction in
softmax: |S| <~ 60 stays inside bf16/fp32 exp range.

Layout: scores are computed directly transposed (S^T = k_sb^T q, keys on
partitions) which kills the per-chunk PE transposes of exp(S); the AV matmul
then produces y^T with a ones-column appended to v^T so the softmax
denominator Z accumulates as column 512 of the same PSUM tile.  y^T is
normalized by 1/Z (per-partition scale), PE-transposed to y, biased with bv
(valid since sum(alpha)=1), and output-projected.

Pipelining: per-sample emission order is proj(s+1) [K,V,Q] -> tail(s)
[normalize, y-transpose, out-proj, DMA out] -> attn(s+1), so the PE fills the
normalize/O latency of sample s with sample s+1's projection work.  PSUM is
exactly 8 banks: 2 rotating accumulators (K/V/Q/O), 2 rotating S^T/transpose
banks, 4 attention-output banks (2 query-tiles x 2 halves, Z in the second
half's col 256... col 512 overall).
"""

import os
import numpy as np

import concourse.bass as bass
import concourse.tile as tile
from concourse import bacc, mybir
from concourse.bass_utils import run_bass_kernel_spmd
from concourse.masks import make_identity

F32 = mybir.dt.float32
F16 = mybir.dt.float16
BF16 = mybir.dt.bfloat16

# Problem shapes (hardcoded per contract)
B, C, T, H, W = 32, 1024, 9, 14, 14
CI = 512
HW = H * W              # 196
THW = T * HW            # 1764
NCORES = 8
BPC = B // NCORES       # 4 samples per core
CT = C // 128           # 8 C-tiles
MI = CI // 128          # 4 Ci-tiles
ITEM0 = (T // 2) * HW   # 784, center-frame column offset
QT = [(0, 128), (128, HW - 128)]                  # query partition tiles
NJ = 4
JW = THW // NJ                                    # 441, K-proj PSUM slice
KT = [(o, min(128, THW - o)) for o in range(0, THW, 128)]  # 14 key subtiles
NKT = len(KT)

REPEAT = int(os.environ.get("KREPEAT", "1"))
IDENT = mybir.ActivationFunctionType.Identity
COPY = mybir.ActivationFunctionType.Copy
EXP = mybir.ActivationFunctionType.Exp


def build_kernel(repeat):
    nc = bacc.Bacc("TRN2", target_bir_lowering=False, debug=False)

    x_d = nc.dram_tensor("x", [BPC, C, THW], F16, kind="ExternalInput")
    wqt_d = nc.dram_tensor("wqt", [C, CI], F16, kind="ExternalInput")
    wkn_d = nc.dram_tensor("wkn", [CI, C], F16, kind="ExternalInput")
    wvt_d = nc.dram_tensor("wvt", [C, CI], F16, kind="ExternalInput")
    wot_d = nc.dram_tensor("wot", [CI, C], F16, kind="ExternalInput")
    bq_d = nc.dram_tensor("bq", [MI, 128], F32, kind="ExternalInput")
    bk_d = nc.dram_tensor("bk", [MI, 128], F32, kind="ExternalInput")
    bv_d = nc.dram_tensor("bv", [MI, 128], F32, kind="ExternalInput")
    bo_d = nc.dram_tensor("bo", [CT, 128], F32, kind="ExternalInput")
    out_d = nc.dram_tensor("out", [BPC, C, HW], F32, kind="ExternalOutput")

    with tile.TileContext(nc) as tc:
        with tc.tile_pool(name="const", bufs=1) as const_pool:
            ident = const_pool.tile([128, 128], F16)
            bq_sb = const_pool.tile([128, MI], F32)
            bk_sb = const_pool.tile([128, MI], F32)
            bv_sb = const_pool.tile([128, MI], F32)
            bo_sb = const_pool.tile([128, CT], F32)

            env = dict(
                nc=nc, tc=tc, x_d=x_d, out_d=out_d,
                wqt_d=wqt_d, wkn_d=wkn_d, wvt_d=wvt_d, wot_d=wot_d,
                bq_d=bq_d, bk_d=bk_d, bv_d=bv_d, bo_d=bo_d,
                ident=ident, bq_sb=bq_sb, bk_sb=bk_sb, bv_sb=bv_sb,
                bo_sb=bo_sb,
            )
            for rep in range(repeat):
                _emit_iteration(env, first=(rep == 0))

    nc.compile()
    return nc


def _emit_iteration(env, first):
    nc, tc = env["nc"], env["tc"]
    x_d, out_d = env["x_d"], env["out_d"]
    ident = env["ident"]
    bq_sb, bk_sb, bv_sb, bo_sb = (env["bq_sb"], env["bk_sb"],
                                  env["bv_sb"], env["bo_sb"])

    w_pool = tc.alloc_tile_pool(name="wp", bufs=1)
    x_pool = tc.alloc_tile_pool(name="xp", bufs=2)
    q_pool = tc.alloc_tile_pool(name="qp", bufs=2)
    v_pool = tc.alloc_tile_pool(name="vp", bufs=30)
    et_pool = tc.alloc_tile_pool(name="etp", bufs=3)
    yt_pool = tc.alloc_tile_pool(name="ytp", bufs=2)
    ys_pool = tc.alloc_tile_pool(name="ysp", bufs=2)
    o_pool = tc.alloc_tile_pool(name="osb", bufs=2)
    z_pool = tc.alloc_tile_pool(name="zp", bufs=4)
    pacc = tc.alloc_tile_pool(name="pacc", bufs=2, space="PSUM")
    pS = tc.alloc_tile_pool(name="pS", bufs=2, space="PSUM")

    # ---- weights + first x: DMA order is the startup lever ----
    wkn = w_pool.tile([128, MI, C], F16, tag="wkn", name="wkn")
    wvt = w_pool.tile([128, CT, CI], F16, tag="wvt", name="wvt")
    wqt = w_pool.tile([128, CT, CI], F16, tag="wqt", name="wqt")
    wot = w_pool.tile([128, MI, C], F16, tag="wot", name="wot")
    nc.scalar.dma_start(
        wkn[:], env["wkn_d"][:].rearrange("(m p) c -> p m c", p=128))

    def dma_x(s):
        x_s = x_pool.tile([128, CT, THW], F16, tag="x", name=f"x{s}")
        for h in range(2):
            t0 = h * (CT // 2)
            nc.sync.dma_start(
                x_s[:, t0:t0 + CT // 2, :],
                x_d[s, t0 * 128:(t0 + CT // 2) * 128, :]
                .rearrange("(t p) w -> p t w", p=128))
        return x_s

    xs = {0: dma_x(0)}
    nc.scalar.dma_start(
        wvt[:], env["wvt_d"][:].rearrange("(t p) e -> p t e", p=128))
    nc.scalar.dma_start(
        wqt[:], env["wqt_d"][:].rearrange("(t p) e -> p t e", p=128))
    nc.scalar.dma_start(
        wot[:], env["wot_d"][:].rearrange("(m p) c -> p m c", p=128))
    if first:
        nc.scalar.dma_start(bq_sb[:], env["bq_d"][:].rearrange("m p -> p m"))
        nc.scalar.dma_start(bk_sb[:], env["bk_d"][:].rearrange("m p -> p m"))
        nc.scalar.dma_start(bv_sb[:], env["bv_d"][:].rearrange("m p -> p m"))
        nc.scalar.dma_start(bo_sb[:], env["bo_d"][:].rearrange("m p -> p m"))
        make_identity(nc, ident[:])

    state = {}

    def proj(s):
        """V, Q, qk projections of sample s (PE + DVE epilogues).

        K-path associativity: S^T = (Wk x)^T q == x^T (Wk^T q), so instead of
        the 925M-MAC K-projection we compute qk = Wk^T q (103M MACs) and
        contract x directly in the scores matmul.  bk drops out: it shifts
        every key score of a query equally, which softmax cancels.
        """
        if s + 1 < BPC:
            xs[s + 1] = dma_x(s + 1)
        x_s = xs[s]
        split = env.get("ps0") is not None and s == 0
        vts = []
        for kj, (ko, ksz) in enumerate(KT):
            vt = v_pool.tile([128, CI + 1], BF16, tag="vt", name=f"vt{s}_{kj}")
            if split:
                # two half-contractions so V groups complete as soon as half
                # of x has landed; merged on DVE
                pv0 = env["ps0"].tile([128, 512], F32, tag="s0",
                                      name=f"pvl{s}_{kj}")
                pv1 = pacc.tile([128, 512], F32, tag="acc", name=f"pvh{s}_{kj}")
                for t in range(CT // 2):
                    nc.tensor.matmul(
                        pv0[:ksz, :], x_s[:, t, ko:ko + ksz], wvt[:, t, :],
                        start=(t == 0), stop=(t == CT // 2 - 1))
                for t in range(CT // 2, CT):
                    nc.tensor.matmul(
                        pv1[:ksz, :], x_s[:, t, ko:ko + ksz], wvt[:, t, :],
                        start=(t == CT // 2), stop=(t == CT - 1))
                tmp = env["kh_pool"].tile([128, 512], F32, tag="kh",
                                          name=f"vh{s}_{kj}")
                nc.vector.tensor_copy(tmp[:ksz, :], pv0[:ksz, :])
                nc.vector.tensor_tensor(
                    vt[:ksz, :CI], tmp[:ksz, :], pv1[:ksz, :],
                    mybir.AluOpType.add)
            else:
                pv = pacc.tile([128, 512], F32, tag="acc", name=f"pv{s}_{kj}")
                for t in range(CT):
                    nc.tensor.matmul(
                        pv[:ksz, :], x_s[:, t, ko:ko + ksz], wvt[:, t, :],
                        start=(t == 0), stop=(t == CT - 1))
                nc.vector.tensor_copy(vt[:ksz, :CI], pv[:ksz, :])
            nc.vector.memset(vt[:ksz, CI:CI + 1], 1.0)
            vts.append(vt)
        q_s = q_pool.tile([128, MI, HW], F16, tag="q", name=f"q{s}")
        for m in range(MI):
            pq = pacc.tile([128, 512], F32, tag="acc", name=f"pq{s}_{m}")
            for t in range(CT):
                nc.tensor.matmul(
                    pq[:, :HW],
                    wqt[:, t, m * 128:(m + 1) * 128],
                    x_s[:, t, ITEM0:ITEM0 + HW],
                    start=(t == 0), stop=(t == CT - 1))
            nc.vector.tensor_scalar_add(
                q_s[:, m, :], pq[:, :HW], bq_sb[:, m:m + 1])
        qk_sb = q_pool.tile([128, CT, HW], F16, tag="qk", name=f"qk{s}")
        for ct in range(CT):
            pqk = pacc.tile([128, 512], F32, tag="acc", name=f"pqk{s}_{ct}")
            for m in range(MI):
                nc.tensor.matmul(
                    pqk[:, :HW],
                    wkn[:, m, ct * 128:(ct + 1) * 128],
                    q_s[:, m, :],
                    start=(m == 0), stop=(m == MI - 1))
            nc.vector.tensor_copy(qk_sb[:, ct, :], pqk[:, :HW])
        state[s] = dict(qk_sb=qk_sb, x_s=x_s, vts=vts)

    def attn(s):
        """S^T scores -> exp -> AV (y^T with Z in col 512 of half b)."""
        st = state[s]
        qk_sb, x_s, vts = st["qk_sb"], st["x_s"], st["vts"]
        py = {}
        for qi in range(len(QT)):
            py[qi, 0] = pY.tile([128, 256], F32, tag="pya", name=f"pya{s}_{qi}")
            py[qi, 1] = pY.tile([128, 257], F32, tag="pyb", name=f"pyb{s}_{qi}")
        ets = [None] * NKT

        def st_group(kj):
            ko, ksz = KT[kj]
            psT = pS.tile([128, 196], F32, tag="ps", name=f"ps{s}_{kj}")
            for ct in range(CT):
                nc.tensor.matmul(
                    psT[:ksz, :],
                    x_s[:, ct, ko:ko + ksz],
                    qk_sb[:, ct, :],
                    start=(ct == 0), stop=(ct == CT - 1))
            et = et_pool.tile([128, HW], BF16, tag="et", name=f"et{s}_{kj}")
            nc.scalar.activation(et[:ksz, :], psT[:ksz, :], EXP)
            ets[kj] = et

        def av_group(kj):
            ko, ksz = KT[kj]
            et = ets[kj]
            for qi, (qo, qsz) in enumerate(QT):
                nc.tensor.matmul(
                    py[qi, 0][:qsz, :],
                    et[:ksz, qo:qo + qsz],
                    vts[kj][:ksz, 0:256],
                    start=(kj == 0), stop=(kj == NKT - 1))
                nc.tensor.matmul(
                    py[qi, 1][:qsz, :],
                    et[:ksz, qo:qo + qsz],
                    vts[kj][:ksz, 256:CI + 1],
                    start=(kj == 0), stop=(kj == NKT - 1))

        for kj in range(NKT):
            st_group(kj)
            if kj >= 1:
                av_group(kj - 1)
        av_group(NKT - 1)
        st["py"] = py

    def tail(s):
        """normalize y^T by 1/Z, transpose to y, add bv, out-proj, DMA."""
        py = state[s]["py"]
        y_s = ys_pool.tile([128, MI, HW], F16, tag="y", name=f"y{s}")
        for qi, (qo, qsz) in enumerate(QT):
            rz = z_pool.tile([128, 1], F32, tag="rz", name=f"rz{s}_{qi}")
            nc.vector.reciprocal(rz[:qsz, :], py[qi, 1][:qsz, 256:257])
            yt = yt_pool.tile([128, CI], F16, tag="yt", name=f"yt{s}_{qi}")
            nc.vector.tensor_scalar_mul(
                yt[:qsz, 0:256], py[qi, 0][:qsz, :], rz[:qsz, :])
            nc.vector.tensor_scalar_mul(
                yt[:qsz, 256:CI], py[qi, 1][:qsz, 0:256], rz[:qsz, :])
            for m in range(MI):
                pT = pS.tile([128, 196], F16, tag="ps", name=f"pt{s}_{qi}_{m}")
                nc.tensor.transpose(
                    pT[:, :qsz], yt[:qsz, m * 128:(m + 1) * 128],
                    ident[:qsz, :qsz])
                nc.vector.tensor_scalar_add(
                    y_s[:, m, qo:qo + qsz], pT[:, :qsz],
                    bv_sb[:, m:m + 1])
        halves = QT if s == BPC - 1 else [(0, HW)]
        out_s = o_pool.tile([128, CT, HW], F32, tag="o", name=f"o{s}")
        for mo in range(CT):
            po = pacc.tile([128, 512], F32, tag="acc", name=f"po{s}_{mo}")
            for (qo, qsz) in halves:
                for m in range(MI):
                    nc.tensor.matmul(
                        po[:, qo:qo + qsz],
                        wot[:, m, mo * 128:(mo + 1) * 128],
                        y_s[:, m, qo:qo + qsz],
                        start=(m == 0), stop=(m == MI - 1))
            nc.vector.tensor_scalar_add(
                out_s[:, mo, :], po[:, :HW], bo_sb[:, mo:mo + 1])
        nc.sync.dma_start(
            out_d[s].rearrange("(mo p) q -> p mo q", p=128), out_s[:])
        del state[s]

    # proj(s+1) sits between attn(s) and tail(s) so the PE fills the
    # normalize latency of sample s with sample s+1's projections.
    env["ps0"] = tc.alloc_tile_pool(name="ps0", bufs=2, space="PSUM")
    env["kh_pool"] = tc.alloc_tile_pool(name="khp", bufs=3)
    proj(0)
    env["ps0"].release()
    env["ps0"] = None
    env["kh_pool"].release()
    pY = tc.alloc_tile_pool(name="pY", bufs=2, space="PSUM")
    attn(0)
    for s in range(1, BPC):
        proj(s)
        tail(s - 1)
        attn(s)
    tail(BPC - 1)

    for p in (pY, pS, pacc, z_pool, o_pool, ys_pool, yt_pool, et_pool,
              v_pool, q_pool, x_pool, w_pool):
        p.release()
    env.pop("ps0", None)


_NC_CACHE = {}


def _get_nc():
    key = REPEAT
    if key not in _NC_CACHE:
        _NC_CACHE[key] = build_kernel(REPEAT)
    return _NC_CACHE[key]


def _make_in_maps(inputs):
    x_flat = np.asarray(inputs["x_window"], np.float32).reshape(B, C, THW)
    x16 = x_flat.astype(np.float16)
    shared = {
        "wqt": np.ascontiguousarray(np.asarray(inputs["Wq"], np.float32).T
                                    .astype(np.float16)),
        "wkn": np.ascontiguousarray(np.asarray(inputs["Wk"], np.float32)
                                    .astype(np.float16)),
        "wvt": np.ascontiguousarray(np.asarray(inputs["Wv"], np.float32).T
                                    .astype(np.float16)),
        "wot": np.ascontiguousarray(np.asarray(inputs["Wo"], np.float32).T
                                    .astype(np.float16)),
        "bq": np.ascontiguousarray(
            np.asarray(inputs["bq"], np.float32).reshape(MI, 128)),
        "bk": np.ascontiguousarray(
            np.asarray(inputs["bk"], np.float32).reshape(MI, 128)),
        "bv": np.ascontiguousarray(
            np.asarray(inputs["bv"], np.float32).reshape(MI, 128)),
        "bo": np.ascontiguousarray(
            np.asarray(inputs["bo"], np.float32).reshape(CT, 128)),
    }
    in_maps = []
    for i in range(NCORES):
        m = dict(shared)
        m["x"] = np.ascontiguousarray(x16[i * BPC:(i + 1) * BPC])
        in_maps.append(m)
    return in_maps


def kernel(x_window, Wq, bq, Wk, bk, Wv, bv, Wo, bo):
    nc = _get_nc()
    in_maps = _make_in_maps(dict(
        x_window=x_window, Wq=Wq, bq=bq, Wk=Wk, bk=bk, Wv=Wv, bv=bv,
        Wo=Wo, bo=bo))
    res = run_bass_kernel_spmd(nc, in_maps, list(range(NCORES)))
    out = np.concatenate([res.results[i]["out"] for i in range(NCORES)],
                         axis=0)
    return out.reshape(B, C, 1, H, W)
